# revision 1
# baseline (speedup 1.0000x reference)
"""DCTResolution2D forward on 8 TRN2 NeuronCores.

Math: for rate_weights-derived masks, the whole reference collapses to
    out[b, c] = P @ x[b, c] @ Q
with P [133, 128] and Q [128, 133] computed on host from rate_weights
(DCT matrices + adaptive-span masks folded together).

Device kernel (per core, data parallel over 2048/8 = 256 slices):
  stage 1: S = matmul(lhsT=X_s, rhs=P^T)  -> S = (P X_s)^T  [j=128, u=133]
  stage 2: O_top = matmul(lhsT=S[:, :128], rhs=Q)  [128, 133]  (rows 0..127)
           bottom 5 rows of GROUP slices batched: one matmul
           lhsT = BB [128, 5*GROUP], rhs = Q -> [5*GROUP, 133]
"""

import numpy as np

H = W = 128
NEW_H = NEW_W = 133
B, C = 32, 64
N_CORES = 8
NSLICE = (B * C) // N_CORES  # 256 slices per core
GROUP = 16  # slices per DMA group

_SMOOTH = 4.0
_MAX_RATE = 2.0
_MIN_RATE = 0.0
_MIN_SHAPE = 1.0


def _dct_mat(n_):
    n = np.arange(n_)[None, :].astype(np.float64)
    k = np.arange(n_)[:, None].astype(np.float64)
    d = np.cos(np.pi * (2 * n + 1) * k / (2 * n_)) * np.sqrt(2.0 / n_)
    d[0] *= 1.0 / np.sqrt(2.0)
    return d


def _compute_pq(rate_weights):
    rw = np.asarray(rate_weights, np.float64)
    cur = np.array([H, W], np.float64)
    min_allowed = np.maximum(
        (np.array([_MIN_SHAPE, _MIN_SHAPE]) - _SMOOTH) / cur,
        np.array([_MIN_RATE, _MIN_RATE]),
    )
    r = np.clip(rw, min_allowed, np.array([_MAX_RATE, _MAX_RATE]))
    crop = cur * r
    vmask = np.clip((_SMOOTH + crop[0] - np.arange(NEW_H)) / _SMOOTH, 0, 1)
    hmask = np.clip((_SMOOTH + crop[1] - np.arange(NEW_W)) / _SMOOTH, 0, 1)
    dh, dw, dh2, dw2 = _dct_mat(H), _dct_mat(W), _dct_mat(NEW_H), _dct_mat(NEW_W)
    p = (dh2[:H, :].T * vmask[None, :H]) @ dh  # [133, 128]
    q = dw.T @ (hmask[:W, None] * dw2[:W, :])  # [128, 133]
    return p.astype(np.float32), q.astype(np.float32)


def _build_nc(nslice=NSLICE, group=GROUP):
    import concourse.bass as bass
    import concourse.tile as tile
    from concourse import bacc, mybir

    f32 = mybir.dt.float32
    nc = bacc.Bacc("TRN2", target_bir_lowering=False, debug=False)

    x = nc.dram_tensor("x", [nslice, H, W], f32, kind="ExternalInput").ap()
    pt = nc.dram_tensor("pt", [H, NEW_H], f32, kind="ExternalInput").ap()
    q = nc.dram_tensor("q", [W, NEW_W], f32, kind="ExternalInput").ap()
    otop = nc.dram_tensor("otop", [nslice, H, NEW_W], f32, kind="ExternalOutput").ap()
    obot = nc.dram_tensor(
        "obot", [nslice, NEW_H - H, NEW_W], f32, kind="ExternalOutput"
    ).ap()

    nbot = NEW_H - H  # 5
    with tile.TileContext(nc) as tc:
        with (
            tc.tile_pool(name="const", bufs=1) as cpool,
            tc.tile_pool(name="xin", bufs=3) as xpool,
            tc.tile_pool(name="mid", bufs=4) as mpool,
            tc.tile_pool(name="bot", bufs=2) as bpool,
            tc.tile_pool(name="out", bufs=2) as opool,
            tc.tile_pool(name="ps1", bufs=2, space="PSUM") as ps1,
            tc.tile_pool(name="ps2", bufs=2, space="PSUM") as ps2,
            tc.tile_pool(name="ps3", bufs=2, space="PSUM") as ps3,
        ):
            pt_sb = cpool.tile([H, NEW_H], f32)
            nc.sync.dma_start(pt_sb[:], pt[:])
            q_sb = cpool.tile([W, NEW_W], f32)
            nc.sync.dma_start(q_sb[:], q[:])

            for g in range(nslice // group):
                sl = slice(g * group, (g + 1) * group)
                xt = xpool.tile([H, group, W], f32)
                nc.sync.dma_start(xt[:], x[sl].rearrange("n p m -> p n m"))
                ot = opool.tile([H, group, NEW_W], f32)
                bb = bpool.tile([W, group * nbot], f32)
                for k in range(group):
                    s_ps = ps1.tile([W, NEW_H], f32)
                    nc.tensor.matmul(s_ps[:], xt[:, k, :], pt_sb[:])
                    s_sb = mpool.tile([W, H], f32)
                    nc.scalar.copy(s_sb[:], s_ps[:, 0:H])
                    nc.vector.tensor_copy(
                        bb[:, k * nbot : (k + 1) * nbot], s_ps[:, H:NEW_H]
                    )
                    o_ps = ps2.tile([H, NEW_W], f32)
                    nc.tensor.matmul(o_ps[:], s_sb[:], q_sb[:])
                    nc.vector.tensor_copy(ot[:, k, :], o_ps[:])
                ob_ps = ps3.tile([group * nbot, NEW_W], f32)
                nc.tensor.matmul(ob_ps[:], bb[:], q_sb[:])
                ob_sb = bpool.tile([group * nbot, NEW_W], f32)
                nc.vector.tensor_copy(ob_sb[:], ob_ps[:])
                nc.sync.dma_start(otop[sl].rearrange("n u v -> u n v"), ot[:])
                nc.sync.dma_start(obot[sl].rearrange("n r v -> (n r) v"), ob_sb[:])

    nc.compile()
    return nc


_CACHE = {}


def _get_nc():
    if "nc" not in _CACHE:
        _CACHE["nc"] = _build_nc()
    return _CACHE["nc"]


def run(x, rate_weights, trace=False):
    """Returns (full_output, BassKernelResults)."""
    from concourse import bass_utils

    p, q = _compute_pq(rate_weights)
    pt = np.ascontiguousarray(p.T)  # [128, 133]
    xs = np.ascontiguousarray(np.asarray(x, np.float32).reshape(B * C, H, W))
    shards = xs.reshape(N_CORES, NSLICE, H, W)
    in_maps = [
        {"x": shards[c], "pt": pt, "q": np.ascontiguousarray(q)}
        for c in range(N_CORES)
    ]
    nc = _get_nc()
    res = bass_utils.run_bass_kernel_spmd(
        nc, in_maps, core_ids=list(range(N_CORES)), trace=trace
    )
    out = np.empty((B * C, NEW_H, NEW_W), np.float32)
    for c in range(N_CORES):
        r = res.results[c]
        lo, hi = c * NSLICE, (c + 1) * NSLICE
        out[lo:hi, :H, :] = r["otop"]
        out[lo:hi, H:, :] = r["obot"]
    return out.reshape(B, C, NEW_H, NEW_W), res


def kernel(x, rate_weights):
    out, _ = run(x, rate_weights)
    return out


# revision 3
# speedup vs baseline: 16452.6742x; 16452.6742x over previous
"""DCTResolution2D forward on 8 TRN2 NeuronCores.

Math: for rate_weights-derived masks, the whole reference collapses to
    out[b, c] = P @ x[b, c] @ Q
with P [133, 128] and Q [128, 133] computed on host from rate_weights
(DCT matrices + adaptive-span masks folded together).

Device kernel (per core, data parallel over 2048/8 = 256 slices):
  stage 1: S = matmul(lhsT=X_s, rhs=P^T)  -> S = (P X_s)^T  [j=128, u=133]
  stage 2: O_top = matmul(lhsT=S[:, :128], rhs=Q)  [128, 133]  (rows 0..127)
           bottom 5 rows of GROUP slices batched: one matmul
           lhsT = BB [128, 5*GROUP], rhs = Q -> [5*GROUP, 133]
"""

import numpy as np

H = W = 128
NEW_H = NEW_W = 133
B, C = 32, 64
N_CORES = 8
NSLICE = (B * C) // N_CORES  # 256 slices per core
GROUP = 16  # slices per DMA group

_SMOOTH = 4.0
_MAX_RATE = 2.0
_MIN_RATE = 0.0
_MIN_SHAPE = 1.0


def _dct_mat(n_):
    n = np.arange(n_)[None, :].astype(np.float64)
    k = np.arange(n_)[:, None].astype(np.float64)
    d = np.cos(np.pi * (2 * n + 1) * k / (2 * n_)) * np.sqrt(2.0 / n_)
    d[0] *= 1.0 / np.sqrt(2.0)
    return d


def _compute_pq(rate_weights):
    rw = np.asarray(rate_weights, np.float64)
    cur = np.array([H, W], np.float64)
    min_allowed = np.maximum(
        (np.array([_MIN_SHAPE, _MIN_SHAPE]) - _SMOOTH) / cur,
        np.array([_MIN_RATE, _MIN_RATE]),
    )
    r = np.clip(rw, min_allowed, np.array([_MAX_RATE, _MAX_RATE]))
    crop = cur * r
    vmask = np.clip((_SMOOTH + crop[0] - np.arange(NEW_H)) / _SMOOTH, 0, 1)
    hmask = np.clip((_SMOOTH + crop[1] - np.arange(NEW_W)) / _SMOOTH, 0, 1)
    dh, dw, dh2, dw2 = _dct_mat(H), _dct_mat(W), _dct_mat(NEW_H), _dct_mat(NEW_W)
    p = (dh2[:H, :].T * vmask[None, :H]) @ dh  # [133, 128]
    q = dw.T @ (hmask[:W, None] * dw2[:W, :])  # [128, 133]
    return p.astype(np.float32), q.astype(np.float32)


def _build_nc(nslice=NSLICE, group=GROUP, passes=1):
    import concourse.bass as bass
    import concourse.tile as tile
    from concourse import bacc, mybir

    f32 = mybir.dt.float32
    nc = bacc.Bacc("TRN2", target_bir_lowering=False, debug=False)

    x = nc.dram_tensor("x", [nslice, H, W], f32, kind="ExternalInput").ap()
    pt = nc.dram_tensor("pt", [H, NEW_H], f32, kind="ExternalInput").ap()
    q = nc.dram_tensor("q", [W, NEW_W], f32, kind="ExternalInput").ap()
    otop = nc.dram_tensor("otop", [nslice, H, NEW_W], f32, kind="ExternalOutput").ap()
    obot = nc.dram_tensor(
        "obot", [nslice, NEW_H - H, NEW_W], f32, kind="ExternalOutput"
    ).ap()

    nbot = NEW_H - H  # 5
    with tile.TileContext(nc) as tc:
        with (
            tc.tile_pool(name="const", bufs=1) as cpool,
            tc.tile_pool(name="xin", bufs=3) as xpool,
            tc.tile_pool(name="mid", bufs=4) as mpool,
            tc.tile_pool(name="bot", bufs=2) as bpool,
            tc.tile_pool(name="out", bufs=2) as opool,
            tc.tile_pool(name="ps1", bufs=2, space="PSUM") as ps1,
            tc.tile_pool(name="ps2", bufs=2, space="PSUM") as ps2,
            tc.tile_pool(name="ps3", bufs=2, space="PSUM") as ps3,
        ):
            pt_sb = cpool.tile([H, NEW_H], f32)
            nc.sync.dma_start(pt_sb[:], pt[:])
            q_sb = cpool.tile([W, NEW_W], f32)
            nc.sync.dma_start(q_sb[:], q[:])

            for g in [gg for _ in range(passes) for gg in range(nslice // group)]:
                sl = slice(g * group, (g + 1) * group)
                xt = xpool.tile([H, group, W], f32)
                nc.sync.dma_start(xt[:], x[sl].rearrange("n p m -> p n m"))
                ot = opool.tile([H, group, NEW_W], f32)
                bb = bpool.tile([W, group * nbot], f32)
                for k in range(group):
                    s_ps = ps1.tile([W, NEW_H], f32)
                    nc.tensor.matmul(s_ps[:], xt[:, k, :], pt_sb[:])
                    s_sb = mpool.tile([W, H], f32)
                    nc.scalar.copy(s_sb[:], s_ps[:, 0:H])
                    nc.vector.tensor_copy(
                        bb[:, k * nbot : (k + 1) * nbot], s_ps[:, H:NEW_H]
                    )
                    o_ps = ps2.tile([H, NEW_W], f32)
                    nc.tensor.matmul(o_ps[:], s_sb[:], q_sb[:])
                    nc.vector.tensor_copy(ot[:, k, :], o_ps[:])
                ob_ps = ps3.tile([group * nbot, NEW_W], f32)
                nc.tensor.matmul(ob_ps[:], bb[:], q_sb[:])
                ob_sb = bpool.tile([group * nbot, NEW_W], f32)
                nc.vector.tensor_copy(ob_sb[:], ob_ps[:])
                nc.sync.dma_start(otop[sl].rearrange("n u v -> u n v"), ot[:])
                nc.sync.dma_start(obot[sl].rearrange("n r v -> (n r) v"), ob_sb[:])

    nc.compile()
    return nc


_CACHE = {}


def _get_nc():
    if "nc" not in _CACHE:
        _CACHE["nc"] = _build_nc()
    return _CACHE["nc"]


def run(x, rate_weights, trace=False):
    """Returns (full_output, BassKernelResults)."""
    from concourse import bass_utils

    p, q = _compute_pq(rate_weights)
    pt = np.ascontiguousarray(p.T)  # [128, 133]
    xs = np.ascontiguousarray(np.asarray(x, np.float32).reshape(B * C, H, W))
    shards = xs.reshape(N_CORES, NSLICE, H, W)
    in_maps = [
        {"x": shards[c], "pt": pt, "q": np.ascontiguousarray(q)}
        for c in range(N_CORES)
    ]
    nc = _get_nc()
    res = bass_utils.run_bass_kernel_spmd(
        nc, in_maps, core_ids=list(range(N_CORES)), trace=trace
    )
    out = np.empty((B * C, NEW_H, NEW_W), np.float32)
    for c in range(N_CORES):
        r = res.results[c]
        lo, hi = c * NSLICE, (c + 1) * NSLICE
        out[lo:hi, :H, :] = r["otop"]
        out[lo:hi, H:, :] = r["obot"]
    return out.reshape(B, C, NEW_H, NEW_W), res


def kernel(x, rate_weights):
    out, _ = run(x, rate_weights)
    return out


# revision 10
# speedup vs baseline: 70843.5758x; 4.3059x over previous
"""DCTResolution2D forward on 8 TRN2 NeuronCores.

Math: for rate_weights-derived masks, the whole reference collapses to
    out[b, c] = P @ x[b, c] @ Q
with P [133, 128] and Q [128, 133] computed on host from rate_weights
(DCT matrices + adaptive-span masks folded together).

Device kernel (per core, data parallel over 2048/8 = 256 slices):
  stage 1: S = matmul(lhsT=X_s, rhs=P^T)  -> S = (P X_s)^T  [j=128, u=133]
  stage 2: O_top = matmul(lhsT=S[:, :128], rhs=Q)  [128, 133]  (rows 0..127)
           bottom 5 rows of GROUP slices batched: one matmul
           lhsT = BB [128, 5*GROUP], rhs = Q -> [5*GROUP, 133]
"""

import numpy as np

H = W = 128
NEW_H = NEW_W = 133
B, C = 32, 64
N_CORES = 8
NSLICE = (B * C) // N_CORES  # 256 slices per core
GROUP = 16  # slices per DMA group

_SMOOTH = 4.0
_MAX_RATE = 2.0
_MIN_RATE = 0.0
_MIN_SHAPE = 1.0


def _dct_mat(n_):
    n = np.arange(n_)[None, :].astype(np.float64)
    k = np.arange(n_)[:, None].astype(np.float64)
    d = np.cos(np.pi * (2 * n + 1) * k / (2 * n_)) * np.sqrt(2.0 / n_)
    d[0] *= 1.0 / np.sqrt(2.0)
    return d


def _compute_pq(rate_weights):
    rw = np.asarray(rate_weights, np.float64)
    cur = np.array([H, W], np.float64)
    min_allowed = np.maximum(
        (np.array([_MIN_SHAPE, _MIN_SHAPE]) - _SMOOTH) / cur,
        np.array([_MIN_RATE, _MIN_RATE]),
    )
    r = np.clip(rw, min_allowed, np.array([_MAX_RATE, _MAX_RATE]))
    crop = cur * r
    vmask = np.clip((_SMOOTH + crop[0] - np.arange(NEW_H)) / _SMOOTH, 0, 1)
    hmask = np.clip((_SMOOTH + crop[1] - np.arange(NEW_W)) / _SMOOTH, 0, 1)
    dh, dw, dh2, dw2 = _dct_mat(H), _dct_mat(W), _dct_mat(NEW_H), _dct_mat(NEW_W)
    p = (dh2[:H, :].T * vmask[None, :H]) @ dh  # [133, 128]
    q = dw.T @ (hmask[:W, None] * dw2[:W, :])  # [128, 133]
    return p.astype(np.float32), q.astype(np.float32)


def _build_nc(nslice=NSLICE, group=GROUP, passes=1, cfg=None):
    cfg = cfg or {}
    b_xin = cfg.get("xin", 3)
    b_mid = cfg.get("mid", 4)
    b_out = cfg.get("out", 2)
    b_ps1 = cfg.get("ps1", 2)
    b_ps2 = cfg.get("ps2", 2)
    import concourse.bass as bass
    import concourse.tile as tile
    from concourse import bacc, mybir

    f32 = mybir.dt.float32
    nc = bacc.Bacc("TRN2", target_bir_lowering=False, debug=False)

    # x is host-pre-permuted to [H, nslice, W] so each partition's DMA run
    # is group*W*4 contiguous bytes; otop likewise [H, nslice, NEW_W].
    x = nc.dram_tensor("x", [H, nslice, W], f32, kind="ExternalInput").ap()
    pt = nc.dram_tensor("pt", [H, NEW_H], f32, kind="ExternalInput").ap()
    q = nc.dram_tensor("q", [W, NEW_W], f32, kind="ExternalInput").ap()
    otop = nc.dram_tensor("otop", [H, nslice, NEW_W], f32, kind="ExternalOutput").ap()
    obot = nc.dram_tensor(
        "obot", [nslice, NEW_H - H, NEW_W], f32, kind="ExternalOutput"
    ).ap()

    nbot = NEW_H - H  # 5
    with tile.TileContext(nc) as tc:
        with (
            tc.tile_pool(name="const", bufs=1) as cpool,
            tc.tile_pool(name="xin", bufs=b_xin) as xpool,
            tc.tile_pool(name="mid", bufs=b_mid) as mpool,
            tc.tile_pool(name="bot", bufs=2) as bpool,
            tc.tile_pool(name="out", bufs=b_out) as opool,
            tc.tile_pool(name="ps1", bufs=b_ps1, space="PSUM") as ps1,
            tc.tile_pool(name="ps2", bufs=b_ps2, space="PSUM") as ps2,
            tc.tile_pool(name="ps3", bufs=cfg.get("ps3", 2), space="PSUM") as ps3,
        ):
            pt_sb = cpool.tile([H, NEW_H], f32)
            nc.sync.dma_start(pt_sb[:], pt[:])
            q_sb = cpool.tile([W, NEW_W], f32)
            nc.sync.dma_start(q_sb[:], q[:])

            for g in [gg for _ in range(passes) for gg in range(nslice // group)]:
                sl = slice(g * group, (g + 1) * group)
                xt = xpool.tile([H, group, W], f32)
                nc.sync.dma_start(xt[:], x[:, sl, :])
                ot = opool.tile([H, group, NEW_W], f32)
                bb = bpool.tile([W, group * nbot], f32)
                for k in range(group):
                    s_ps = ps1.tile([W, NEW_H], f32)
                    nc.tensor.matmul(s_ps[:], xt[:, k, :], pt_sb[:])
                    s_sb = mpool.tile([W, H], f32)
                    nc.scalar.copy(s_sb[:], s_ps[:, 0:H])
                    nc.vector.tensor_copy(
                        bb[:, k * nbot : (k + 1) * nbot], s_ps[:, H:NEW_H]
                    )
                    o_ps = ps2.tile([H, NEW_W], f32)
                    nc.tensor.matmul(o_ps[:], s_sb[:], q_sb[:])
                    nc.vector.tensor_copy(ot[:, k, :], o_ps[:])
                ob_ps = ps3.tile([group * nbot, NEW_W], f32)
                nc.tensor.matmul(ob_ps[:], bb[:], q_sb[:])
                ob_sb = bpool.tile([group * nbot, NEW_W], f32)
                nc.vector.tensor_copy(ob_sb[:], ob_ps[:])
                nc.sync.dma_start(otop[:, sl, :], ot[:])
                nc.sync.dma_start(obot[sl].rearrange("n r v -> (n r) v"), ob_sb[:])

    nc.compile()
    return nc


_CACHE = {}


def _get_nc():
    if "nc" not in _CACHE:
        _CACHE["nc"] = _build_nc()
    return _CACHE["nc"]


def run(x, rate_weights, trace=False):
    """Returns (full_output, BassKernelResults)."""
    from concourse import bass_utils

    p, q = _compute_pq(rate_weights)
    pt = np.ascontiguousarray(p.T)  # [128, 133]
    xs = np.asarray(x, np.float32).reshape(N_CORES, NSLICE, H, W)
    # per-core permute to [H, NSLICE, W] so device DMA runs are contiguous
    shards = np.ascontiguousarray(xs.transpose(0, 2, 1, 3))
    in_maps = [
        {"x": shards[c], "pt": pt, "q": np.ascontiguousarray(q)}
        for c in range(N_CORES)
    ]
    nc = _get_nc()
    res = bass_utils.run_bass_kernel_spmd(
        nc, in_maps, core_ids=list(range(N_CORES)), trace=trace
    )
    out = np.empty((B * C, NEW_H, NEW_W), np.float32)
    for c in range(N_CORES):
        r = res.results[c]
        lo, hi = c * NSLICE, (c + 1) * NSLICE
        out[lo:hi, :H, :] = r["otop"].transpose(1, 0, 2)
        out[lo:hi, H:, :] = r["obot"]
    return out.reshape(B, C, NEW_H, NEW_W), res


def kernel(x, rate_weights):
    out, _ = run(x, rate_weights)
    return out
